# revision 1
# baseline (speedup 1.0000x reference)
"""Trainium2 Bass kernel for nn_Encoding3D (vq_codebook).

Math: for each voxel feature x = X[b,d,n] (N = T*H*W):
    logit_k = scale[k,d] * (x - cw[k,d])^2 = a*x^2 + b*x + c   (a=s, b=-2sc, c=sc^2)
    A = softmax_k(logit)
    E[b,n,d] = sum_k A_k * (x - cw_k) = x - (sum_k e_k*cw_k)/(sum_k e_k)
    E_glob[b,d] = (1/K) * sum_n E;  gamma = sigmoid(E_glob @ fc_w.T + fc_b)
    out = relu(E * (1 + gamma))

Sharding: 8 cores = (b in 0..3) x (N-half in 0..1); the only cross-core
reduction is sum_n E (64 floats) -> AllReduce over core pairs.

Per-core pipeline (4096 voxels, chunks of 1024, 16 channel-groups of 4):
  PE:  logits[(d4,k)=128, n] = coefT_g.T @ basis. The fp16 basis holds, per
       16-channel set, rows [u_hi | u_lo | u_hi*2^-11 | v_hi] (u = x^2,
       v = x), paired with fp16 weights [a_hi | a_hi | a_lo*2^11 | b_hi] --
       a split-precision product giving ~1e-4 absolute logit accuracy.
  ACT: e = Exp(logits + cbias_g)  (cbias = s*c^2 + t_d; t_d = per-channel
       softmax-invariant shift keeping e in fp16 range)  -> fp16 SBUF
  PE:  sums[128, n] += selT_g.T @ e   (s0_d rows 0..63, s1_d rows 64..127)
  DVE: E = x - s1 * recip(s0); accumulate sum_n E
  tail: pairwise AllGather(sum_n E, 256B) -> gamma -> out = relu(E*(1+gamma))

The (c, g) stream is software-pipelined with a 2-group skew and a PE
warm-up burst so the tensor engine stays at 2.4 GHz (HAM un-throttled).
"""

import numpy as np

import concourse.bacc as bacc
import concourse.bass as bass
import concourse.mybir as mybir
import concourse.tile as tile
from concourse.bass_utils import run_bass_kernel_spmd

B, D, K = 4, 64, 32
T, H, W = 8, 32, 32
N = T * H * W            # 8192
NCORES = 8
NL = N // 2              # 4096 voxels per core
CH = 1024                # chunk (free-dim) size
NCH = NL // CH           # 4 chunks
NG = D // 4              # 16 groups of 4 channels
f32 = mybir.dt.float32
f16 = mybir.dt.float16

AF = mybir.ActivationFunctionType
ALU = mybir.AluOpType


def _build_nc(use_collective=True, dbg=False):
    nc = bacc.Bacc("TRN2", target_bir_lowering=False, debug=False,
                   num_devices=NCORES if use_collective else 1)

    x_d = nc.dram_tensor("x", [D, NL], f32, kind="ExternalInput")
    coefT_d = nc.dram_tensor("coefT", [128, 128 * NG], f16, kind="ExternalInput")
    selT_d = nc.dram_tensor("selT", [128, 128 * NG], f16, kind="ExternalInput")
    cbias_d = nc.dram_tensor("cbias", [128, NG], f32, kind="ExternalInput")
    fcwT_d = nc.dram_tensor("fcwT", [D, D], f32, kind="ExternalInput")
    nfcb_d = nc.dram_tensor("nfcb", [D, 1], f32, kind="ExternalInput")
    wrm_d = nc.dram_tensor("wrm", [128, 512], f16, kind="ExternalInput")
    out_d = nc.dram_tensor("out", [D, NL], f32, kind="ExternalOutput")
    if dbg:
        dbgE_d = nc.dram_tensor("dbgE", [D, NL], f32, kind="ExternalOutput")
        dbgS_d = nc.dram_tensor("dbgS", [128, CH], f32, kind="ExternalOutput")
        dbge_d = nc.dram_tensor("dbge", [128, CH], f16, kind="ExternalOutput")
        dbgB_d = nc.dram_tensor("dbgB", [128, CH], f16, kind="ExternalOutput")
        dbgG_d = nc.dram_tensor("dbgG", [D, 1], f32, kind="ExternalOutput")

    with tile.TileContext(nc) as tc:
        with (
            tc.tile_pool(name="const", bufs=1) as cpool,
            tc.tile_pool(name="flat", bufs=2) as fpool,
            tc.tile_pool(name="basis", bufs=2) as bpool,
            tc.tile_pool(name="ework", bufs=3) as epool,
            tc.tile_pool(name="fin", bufs=2) as finpool,
            tc.tile_pool(name="persist", bufs=1) as ppool,
            tc.tile_pool(name="psumL", bufs=3, space=bass.MemorySpace.PSUM) as psL,
            tc.tile_pool(name="psumS", bufs=1, space=bass.MemorySpace.PSUM) as psS,
            tc.tile_pool(name="dram", bufs=1, space="DRAM") as dpool,
        ):
            coefT = cpool.tile([128, 128 * NG], f16, tag="coefT")
            selT = cpool.tile([128, 128 * NG], f16, tag="selT")
            cbias = cpool.tile([128, NG], f32, tag="cbias")
            fcwT = cpool.tile([D, D], f32, tag="fcwT")
            nfcb = cpool.tile([D, 1], f32, tag="nfcb")
            xt = ppool.tile([D, NL], f32, tag="xt")
            wrm = cpool.tile([128, 512], f16, tag="wrm")
            # warm-up const first (tiny), then x chunk 0 split across both
            # queues (critical path); consts on gpsimd; basis scatter
            # alternates sync/gpsimd (~0.6us issue per dma_start per queue)
            nc.sync.dma_start(wrm[:], wrm_d[:])
            TH = CH // 3
            nc.sync.dma_start(xt[:, 0:TH], x_d[:, 0:TH])
            nc.gpsimd.dma_start(xt[:, TH:2 * TH], x_d[:, TH:2 * TH])
            nc.scalar.dma_start(xt[:, 2 * TH:CH], x_d[:, 2 * TH:CH])
            nc.gpsimd.dma_start(cbias[:], cbias_d[:])
            nc.gpsimd.dma_start(coefT[:], coefT_d[:])
            for cc_ in range(1, NCH):
                nc.sync.dma_start(xt[:, cc_ * CH:(cc_ + 1) * CH],
                                  x_d[:, cc_ * CH:(cc_ + 1) * CH])
            nc.gpsimd.dma_start(selT[:], selT_d[:])
            nc.gpsimd.dma_start(fcwT[:], fcwT_d[:])
            nc.gpsimd.dma_start(nfcb[:], nfcb_d[:])

            Et = ppool.tile([D, NL], f32, tag="Et")
            egp = ppool.tile([D, NCH], f32, tag="egp")

            # PE warm-up: dense dummy matmuls while input DMAs run, so the
            # HAM clock gate reaches 2.4 GHz before the real pipeline starts
            # (idle/cold PE runs matmuls at 1.2 GHz). Uses the sums-pool
            # slot, released before the first real sums accumulation.
            warm = psS.tile([128, 512], f32, tag="sums", name="warm")
            for _ in range(20):
                nc.tensor.matmul(warm[:], wrm[:, 0:128], wrm[:],
                                 start=True, stop=True)

            def basis_prep(c):
                c0 = c * CH
                # ---- per-chunk basis build ----
                vhi = fpool.tile([D, CH], f16, tag="vhi")
                nc.vector.tensor_copy(vhi[:], xt[:, c0:c0 + CH])
                U = fpool.tile([D, CH], f32, tag="U")
                nc.vector.tensor_tensor(U[:], xt[:, c0:c0 + CH],
                                        xt[:, c0:c0 + CH], ALU.mult)
                uhi = fpool.tile([D, CH], f16, tag="uhi")
                nc.vector.tensor_copy(uhi[:], U[:])
                ulo = fpool.tile([D, CH], f16, tag="ulo")
                # ulo = (uhi * -1) + U
                nc.vector.scalar_tensor_tensor(ulo[:], uhi[:], -1.0, U[:],
                                               ALU.mult, ALU.add)
                # uhs = uhi * 2^-11 (exact; pairs with weight a_lo*2^11)
                uhs = fpool.tile([D, CH], f16, tag="uhs")
                nc.vector.tensor_scalar_mul(uhs[:], uhi[:], 2.0 ** -11)

                # basis tile t, 16-ch set s (=2t+s2): rows 64*s2+[0:16) u_hi,
                # [16:32) u_lo, [32:48) uhs, [48:64) v_hi  (channels 16s..16s+15)
                btiles = []
                for t in range(2):
                    bt = bpool.tile([128, CH], f16, tag=f"b{t}")
                    btiles.append(bt)
                    for s2 in range(2):
                        s = 2 * t + s2
                        rb = 64 * s2
                        for q, src in enumerate((uhi, ulo, uhs, vhi)):
                            eng = nc.sync if q % 2 == 0 else nc.gpsimd
                            eng.dma_start(
                                bt[rb + 16 * q:rb + 16 * (q + 1), :],
                                src[16 * s:16 * (s + 1), :])
                return btiles

            # software-pipelined (c, g) stream with 2-group skew: PE always
            # has two groups of logits matmuls queued ahead of the current
            # group's sums matmul, so it never idles waiting on ACT (idle
            # gaps re-throttle the PE clock to 1.2 GHz).
            basis = {0: basis_prep(0)}
            sums_t = {}
            # group order alternates the 64-row basis window (0/64) so
            # adjacent groups' logits matmuls hit different PE row strips
            seq = [0, 4, 1, 5, 2, 6, 3, 7, 8, 12, 9, 13, 10, 14, 11, 15]

            def mm1(c, g):
                s = g // 4
                t, rb = s // 2, 64 * (s % 2)
                logits = psL.tile([128, CH], f32, tag="logits")
                for h in range(CH // 512):
                    nc.tensor.matmul(
                        logits[:, 512 * h:512 * (h + 1)],
                        coefT[rb:rb + 64, 128 * g:128 * (g + 1)],
                        basis[c][t][rb:rb + 64, 512 * h:512 * (h + 1)],
                        start=True, stop=True, tile_position=(rb, 0))
                return logits

            def finals(c):
                sums = sums_t.pop(c)
                if dbg and c == 0:
                    scp = finpool.tile([128, CH], f32, tag="dbgscp")
                    nc.vector.tensor_copy(scp[:], sums[:])
                    nc.sync.dma_start(dbgS_d[:], scp[:])
                # drain PSUM with one fast copy so the next chunk's matmul
                # accumulation can reuse the bank; then finish E off SBUF.
                c0 = c * CH
                # drain s1 to SBUF + reciprocal of s0: after these two the
                # PSUM bank is free for the next chunk's accumulation
                r = finpool.tile([D, CH], f32, tag="recip")
                nc.vector.reciprocal_approx_fast(r[:], sums[0:D, :])
                s1c = finpool.tile([D, CH], f32, tag="s1c")
                nc.vector.tensor_copy(s1c[:], sums[D:128, :])
                corr = finpool.tile([D, CH], f32, tag="corr")
                nc.vector.tensor_tensor(corr[:], s1c[:], r[:], ALU.mult)
                nc.vector.scalar_tensor_tensor(
                    Et[:, c0:c0 + CH], corr[:], -1.0, xt[:, c0:c0 + CH],
                    ALU.mult, ALU.add,
                    accum_out=egp[:, c:c + 1])

            units = [(c, g) for c in range(NCH) for g in seq]
            logits_t = {units[0]: mm1(*units[0]), units[1]: mm1(*units[1])}
            for i, (c, g) in enumerate(units):
                if g == seq[0]:
                    sums_t[c] = psS.tile([128, CH], f32, tag="sums",
                                         name=f"sums{c}")
                if g == seq[6] and c + 1 < NCH:
                    basis[c + 1] = basis_prep(c + 1)
                et = epool.tile([128, CH], f16, tag="et")
                nc.scalar.activation(et[:], logits_t.pop((c, g))[:], AF.Exp,
                                     bias=cbias[:, g:g + 1], scale=1.0)
                if dbg and c == 0 and g == 0:
                    nc.sync.dma_start(dbge_d[:], et[:])
                    nc.sync.dma_start(dbgB_d[:], basis[0][0][:])
                if i + 2 < len(units):
                    logits_t[units[i + 2]] = mm1(*units[i + 2])
                for h in range(CH // 512):
                    nc.tensor.matmul(
                        sums_t[c][:, 512 * h:512 * (h + 1)],
                        selT[:, 128 * g:128 * (g + 1)],
                        et[:, 512 * h:512 * (h + 1)],
                        start=(g == seq[0]), stop=(g == seq[-1]),
                        skip_group_check=True)
                if g == seq[-1]:
                    finals(c)

            # ---- tail: gamma ----
            S = ppool.tile([D, 1], f32, tag="S")
            nc.vector.tensor_reduce(S[:], egp[:, :], mybir.AxisListType.X, ALU.add)
            cc_in = dpool.tile([D, 1], f32, tag="cc_in")
            cc_out2 = dpool.tile([D, 1], f32, tag="cc_out2")
            nc.sync.dma_start(cc_in[:], S[:])
            Sf = ppool.tile([D, 1], f32, tag="Sf")
            if use_collective:
                nc.gpsimd.collective_compute(
                    "AllReduce", ALU.add,
                    replica_groups=[[0, 1], [2, 3], [4, 5], [6, 7]],
                    ins=[cc_in.opt()], outs=[cc_out2.opt()])
                nc.sync.dma_start(Sf[:], cc_out2[:])
            else:
                nc.sync.dma_start(Sf[:], cc_in[:])

            gz = psS.tile([D, 1], f32, tag="sums")
            nc.tensor.matmul(gz[:], fcwT[:], Sf[:], start=True, stop=True)
            ue = ppool.tile([D, 1], f32, tag="ue")
            # ue = exp(-(z) - fcb)
            nc.scalar.activation(ue[:], gz[:], AF.Exp, bias=nfcb[:, 0:1],
                                 scale=-1.0)
            w1 = ppool.tile([D, 1], f32, tag="w1")
            nc.vector.tensor_scalar_add(w1[:], ue[:], 1.0)
            sg = ppool.tile([D, 1], f32, tag="sg")
            nc.vector.reciprocal(sg[:], w1[:])
            g1 = ppool.tile([D, 1], f32, tag="g1")
            nc.vector.tensor_scalar_add(g1[:], sg[:], 1.0)

            # final out = relu(E * (1+gamma)) split across DVE and ACT with
            # one output DMA per quarter so store overlaps compute
            outt = ppool.tile([D, NL], f32, tag="outt")
            HL = NL // 2
            nc.vector.tensor_scalar(outt[:, 0:HL], Et[:, 0:HL], g1[:, 0:1],
                                    0.0, ALU.mult, ALU.max)
            nc.scalar.activation(outt[:, HL:NL], Et[:, HL:NL], AF.Relu,
                                 scale=g1[:, 0:1])
            for q in range(4):
                eng = nc.sync if q % 2 == 0 else nc.scalar
                qs = slice(q * NL // 4, (q + 1) * NL // 4)
                eng.dma_start(out_d[:, qs], outt[:, qs])
            if dbg:
                nc.sync.dma_start(dbgE_d[:], Et[:])
                nc.sync.dma_start(dbgG_d[:], g1[:])

    nc.compile()
    return nc


def _round8_up(v):
    return np.ceil(np.asarray(v) * 8.0) / 8.0


def _prep_inputs(X, codewords, scale, fc_w, fc_b):
    X = np.ascontiguousarray(np.asarray(X, np.float32))
    cw = np.asarray(codewords, np.float64)
    sc = np.asarray(scale, np.float64)

    a32 = sc.astype(np.float32)
    a_hi = a32.astype(np.float16)
    a_lo = (a32 - a_hi.astype(np.float32)).astype(np.float16)
    b_hi = (-2.0 * sc * cw).astype(np.float32).astype(np.float16)
    cterm = (sc * cw * cw).astype(np.float32)

    # per-channel softmax-invariant shift: keeps max_k exp() >= ~1 in fp16
    smin = np.maximum(-sc.max(axis=0), 0.0)           # (D,) min_k |scale|
    t_d = np.minimum(10.0, _round8_up(30.0 * smin)).astype(np.float32)

    cbias = np.zeros((128, NG), np.float32)
    coefT = np.zeros((128, 128 * NG), np.float16)
    selT = np.zeros((128, 128 * NG), np.float16)
    cw_h = cw.astype(np.float32).astype(np.float16)
    a_lo_s = (a_lo.astype(np.float32) * 2.0 ** 11).astype(np.float16)
    for g in range(NG):
        s, j = g // 4, g % 4
        rb = 64 * (s % 2)
        for di in range(4):
            d = 16 * s + 4 * j + di
            m = 128 * g + 32 * di + np.arange(K)
            coefT[rb + 4 * j + di, m] = a_hi[:, d]
            coefT[rb + 16 + 4 * j + di, m] = a_hi[:, d]      # pairs u_lo
            coefT[rb + 32 + 4 * j + di, m] = a_lo_s[:, d]    # pairs uhs
            coefT[rb + 48 + 4 * j + di, m] = b_hi[:, d]      # pairs v_hi
            cbias[32 * di + np.arange(K), g] = cterm[:, d] + t_d[d]
            selT[32 * di + np.arange(K), 128 * g + d] = 1.0
            selT[32 * di + np.arange(K), 128 * g + 64 + d] = cw_h[:, d]

    fcwT = np.ascontiguousarray(
        (np.asarray(fc_w, np.float64).T / K).astype(np.float32))
    nfcb = (-np.asarray(fc_b, np.float64)).astype(np.float32).reshape(D, 1)

    Xf = X.reshape(B, D, N)
    in_maps = []
    for core in range(NCORES):
        b, h = core // 2, core % 2
        in_maps.append({
            "x": np.ascontiguousarray(Xf[b, :, h * NL:(h + 1) * NL]),
            "coefT": coefT,
            "selT": selT,
            "cbias": cbias,
            "fcwT": fcwT,
            "nfcb": nfcb,
            "wrm": np.full((128, 512), 0.5, np.float16),
        })
    return in_maps


_NC = None


def _get_nc():
    global _NC
    if _NC is None:
        _NC = _build_nc()
    return _NC


def run_sharded(X, codewords, scale, fc_w, fc_b, **spmd_kwargs):
    """Build+run; returns (full_output, BassKernelResults)."""
    nc = _get_nc()
    in_maps = _prep_inputs(X, codewords, scale, fc_w, fc_b)
    res = run_bass_kernel_spmd(nc, in_maps, core_ids=list(range(NCORES)),
                               **spmd_kwargs)
    Y = np.empty((B, D, N), np.float32)
    for core in range(NCORES):
        b, h = core // 2, core % 2
        Y[b, :, h * NL:(h + 1) * NL] = res.results[core]["out"]
    return Y.reshape(B, D, T, H, W), res


def kernel(X, codewords, scale, fc_w, fc_b):
    Y, _ = run_sharded(X, codewords, scale, fc_w, fc_b)
    return Y



# revision 4
# speedup vs baseline: 1.2318x; 1.2318x over previous
"""Trainium2 Bass kernel for nn_Encoding3D (vq_codebook).

Math per voxel feature x = X[b,d,n] (N = T*H*W):
    logit_k = scale[k,d]*(x - cw[k,d])^2 = a*u + b*v + c   (u=x^2, v=x,
              a=s, b=-2sc, c=sc^2)
    e_k = exp(logit_k + t_d)   (t_d = per-channel softmax-invariant shift)
    E[b,n,d] = x - (sum_k e_k*cw[k,d]) / (sum_k e_k)
    E_glob[b,d] = (1/K) sum_n E;  gamma = sigmoid(E_glob @ fc_w.T + fc_b)
    out = relu(E) * (1 + gamma)          [1+gamma > 0]

Sharding: 8 cores = (b in 0..3) x (N-half in 0..1); the only cross-core
reduction is sum_n E (64 floats) -> AllReduce over core pairs.

Per-core pipeline (4096 voxels, 4 chunks of 1024, 16 channel-groups of 4):
  DVE: basis bt[128, CH] f16 = [v(0:64); u(64:128)] (x is DMA'd into both
       partition halves of xt2 so no partition-shifted engine writes)
  PE:  logits[(4d,k)=128, 512]x2 = coefT_g.T @ bt  (contract all 128 rows;
       coefT has b at row d, a at row 64+d for that column's channel)
  ACT: e = Exp(logits + cbias_g) -> fp8e4m3, written into the paired
       [g|g+1] layout for DoubleRow
  PE:  sums[128, 512]x2 += selT_pair.T @ e  (fp8 DoubleRow: 2 groups per
       matmul at 0.5 cyc/row; s0_d rows 0..63, s1_d rows 64..127)
  DVE: E = x - s1*recip(s0)  (f16, accum egp) ; tail: AllReduce(sum_n E,
       256B) -> gamma -> out = relu(E)*(1+gamma) -> f16 DMA out
"""

import numpy as np
import ml_dtypes

import concourse.bacc as bacc
import concourse.bass as bass
import concourse.mybir as mybir
import concourse.tile as tile
from concourse.bass_utils import run_bass_kernel_spmd

B, D, K = 4, 64, 32
T, H, W = 8, 32, 32
N = T * H * W            # 8192
NCORES = 8
NL = N // 2              # 4096 voxels per core
CH = 1024                # chunk (free-dim) size
NCH = NL // CH           # 4 chunks
NG = D // 4              # 16 groups of 4 channels
f32 = mybir.dt.float32
f16 = mybir.dt.float16
f8 = mybir.dt.float8e4

AF = mybir.ActivationFunctionType
ALU = mybir.AluOpType
DR = mybir.MatmulPerfMode.DoubleRow

TCAP = 5.0               # keeps e = exp(logit+t) <= e^5.x < 240 (fp8 max)


def _build_nc(use_collective=True):
    nc = bacc.Bacc("TRN2", target_bir_lowering=False, debug=False,
                   num_devices=NCORES if use_collective else 1)

    x_d = nc.dram_tensor("x", [D, NL], f32, kind="ExternalInput")
    coefT_d = nc.dram_tensor("coefT", [128, 128 * NG], f16, kind="ExternalInput")
    selT_d = nc.dram_tensor("selT", [128, 128 * NG], f8, kind="ExternalInput")
    cbias_d = nc.dram_tensor("cbias", [128, NG], f32, kind="ExternalInput")
    fcwT_d = nc.dram_tensor("fcwT", [D, D], f32, kind="ExternalInput")
    nfcb_d = nc.dram_tensor("nfcb", [D, 1], f32, kind="ExternalInput")
    out_d = nc.dram_tensor("out", [D, NL], f16, kind="ExternalOutput")

    with tile.TileContext(nc) as tc:
        with (
            tc.tile_pool(name="const", bufs=1) as cpool,
            tc.tile_pool(name="basis", bufs=2) as bpool,
            tc.tile_pool(name="ework", bufs=2) as epool,
            tc.tile_pool(name="fin", bufs=2) as finpool,
            tc.tile_pool(name="persist", bufs=1) as ppool,
            tc.tile_pool(name="psumL", bufs=3, space=bass.MemorySpace.PSUM) as psL,
            tc.tile_pool(name="psumS", bufs=1, space=bass.MemorySpace.PSUM) as psS,
            tc.tile_pool(name="dram", bufs=1, space="DRAM") as dpool,
        ):
            coefT = cpool.tile([128, 128 * NG], f16, tag="coefT")
            selT = cpool.tile([128, 128 * NG], f8, tag="selT")
            cbias = cpool.tile([128, NG], f32, tag="cbias")
            fcwT = cpool.tile([D, D], f32, tag="fcwT")
            nfcb = cpool.tile([D, 1], f32, tag="nfcb")
            wrm = cpool.tile([128, 512], f16, tag="wrm")
            xt2 = ppool.tile([128, NL], f32, tag="xt2")

            # x chunk 0 into both partition halves first (critical path),
            # split across queues; consts interleaved; rest of x after.
            TH3 = CH // 2
            nc.sync.dma_start(xt2[0:D, 0:TH3], x_d[:, 0:TH3])
            nc.gpsimd.dma_start(xt2[0:D, TH3:CH], x_d[:, TH3:CH])
            nc.scalar.dma_start(xt2[D:128, 0:TH3], x_d[:, 0:TH3])
            nc.sync.dma_start(xt2[D:128, TH3:CH], x_d[:, TH3:CH])
            nc.gpsimd.dma_start(cbias[:], cbias_d[:])
            nc.sync.dma_start(coefT[:], coefT_d[:])
            nc.gpsimd.dma_start(selT[:], selT_d[:])
            for cc_ in range(1, NCH):
                sl = slice(cc_ * CH, (cc_ + 1) * CH)
                nc.sync.dma_start(xt2[0:D, sl], x_d[:, sl])
                nc.gpsimd.dma_start(xt2[D:128, sl], x_d[:, sl])
            nc.scalar.dma_start(fcwT[:], fcwT_d[:])
            nc.scalar.dma_start(nfcb[:], nfcb_d[:])

            Et = ppool.tile([D, NL], f16, tag="Et")
            egp = ppool.tile([D, NCH], f32, tag="egp")

            # PE warm-up: dummy matmuls so the clock leaves the idle p-state
            # before the real pipeline starts.
            nc.vector.memset(wrm[:], 0.5)
            warm = psL.tile([128, CH], f32, tag="logits", name="warm")
            for _ in range(10):
                nc.tensor.matmul(warm[:, 0:512], wrm[:, 0:128], wrm[:],
                                 start=True, stop=True)

            def basis_prep(c):
                c0 = c * CH
                bt = bpool.tile([128, CH], f16, tag="bt")
                nc.vector.tensor_copy(bt[0:D, :], xt2[0:D, c0:c0 + CH])
                nc.vector.tensor_tensor(bt[D:128, :], xt2[D:128, c0:c0 + CH],
                                        xt2[D:128, c0:c0 + CH], ALU.mult)
                return bt

            def mm1(c, g):
                logits = psL.tile([128, CH], f32, tag="logits")
                for h in range(2):
                    nc.tensor.matmul(
                        logits[:, 512 * h:512 * (h + 1)],
                        coefT[:, 128 * g:128 * (g + 1)],
                        basis[c][:, 512 * h:512 * (h + 1)],
                        start=True, stop=True)
                return logits

            def finals(c):
                sums = sums_t.pop(c)
                c0 = c * CH
                r = finpool.tile([D, CH], f32, tag="recip")
                nc.vector.reciprocal_approx_fast(r[:], sums[0:D, :])
                corr = finpool.tile([D, CH], f32, tag="corr")
                nc.vector.tensor_tensor(corr[:], sums[D:128, :], r[:], ALU.mult)
                nc.vector.scalar_tensor_tensor(
                    Et[:, c0:c0 + CH], corr[:], -1.0, xt2[0:D, c0:c0 + CH],
                    ALU.mult, ALU.add,
                    accum_out=egp[:, c:c + 1])

            basis = {0: basis_prep(0)}
            sums_t = {}
            units = [(c, g) for c in range(NCH) for g in range(NG)]
            logits_t = {units[0]: mm1(*units[0]), units[1]: mm1(*units[1])}
            et_t = {}
            for i, (c, g) in enumerate(units):
                if g == 0:
                    sums_t[c] = psS.tile([128, CH], f32, tag="sums",
                                         name=f"sums{c}")
                if g == 8 and c + 1 < NCH:
                    basis[c + 1] = basis_prep(c + 1)
                if g % 2 == 0:
                    # paired e layout [128, (h, j, n)]: j = group within pair
                    et_t[c] = epool.tile([128, 2, 2, 512], f8, tag="et",
                                         name=f"et{c}_{g}")
                et4 = et_t[c]
                nc.scalar.activation(et4[:, :, g % 2, :],
                                     logits_t.pop((c, g))[:], AF.Exp,
                                     bias=cbias[:, g:g + 1], scale=1.0)
                if i + 2 < len(units):
                    logits_t[units[i + 2]] = mm1(*units[i + 2])
                if g % 2 == 1:
                    p = g // 2
                    selv = selT[:, 256 * p:256 * (p + 1)].rearrange(
                        "p (two m) -> p two m", two=2)
                    for h in range(2):
                        nc.tensor.matmul(
                            sums_t[c][:, 512 * h:512 * (h + 1)],
                            selv,
                            et4[:, h, :, :],
                            start=(g == 1), stop=(g == NG - 1),
                            perf_mode=DR,
                            skip_group_check=True)
                if g == NG - 1:
                    finals(c)

            # ---- tail: gamma ----
            S = ppool.tile([D, 1], f32, tag="S")
            nc.vector.tensor_reduce(S[:], egp[:, :], mybir.AxisListType.X, ALU.add)
            cc_in = dpool.tile([D, 1], f32, tag="cc_in")
            cc_out2 = dpool.tile([D, 1], f32, tag="cc_out2")
            nc.sync.dma_start(cc_in[:], S[:])
            Sf = ppool.tile([D, 1], f32, tag="Sf")
            if use_collective:
                nc.gpsimd.collective_compute(
                    "AllReduce", ALU.add,
                    replica_groups=[[0, 1], [2, 3], [4, 5], [6, 7]],
                    ins=[cc_in.opt()], outs=[cc_out2.opt()])
                nc.sync.dma_start(Sf[:], cc_out2[:])
            else:
                nc.sync.dma_start(Sf[:], cc_in[:])

            gz = psS.tile([D, 1], f32, tag="sums", name="gz")
            nc.tensor.matmul(gz[:], fcwT[:], Sf[:], start=True, stop=True)
            ue = ppool.tile([D, 1], f32, tag="ue")
            # ue = exp(-z - fcb); gamma = 1/(1+ue)
            nc.scalar.activation(ue[:], gz[:], AF.Exp, bias=nfcb[:, 0:1],
                                 scale=-1.0)
            w1 = ppool.tile([D, 1], f32, tag="w1")
            nc.vector.tensor_scalar_add(w1[:], ue[:], 1.0)
            sg = ppool.tile([D, 1], f32, tag="sg")
            nc.vector.reciprocal(sg[:], w1[:])
            g1 = ppool.tile([D, 1], f32, tag="g1")
            nc.vector.tensor_scalar_add(g1[:], sg[:], 1.0)

            # out = relu(E)*(1+gamma), split DVE/ACT per quarter with one
            # output DMA per quarter so store overlaps compute
            outt = ppool.tile([D, NL], f16, tag="outt")
            for q in range(4):
                qs = slice(q * NL // 4, (q + 1) * NL // 4)
                if q % 2 == 0:
                    nc.vector.tensor_scalar(outt[:, qs], Et[:, qs],
                                            g1[:, 0:1], 0.0,
                                            ALU.mult, ALU.max)
                else:
                    nc.scalar.activation(outt[:, qs], Et[:, qs], AF.Relu,
                                         scale=g1[:, 0:1])
                eng = nc.sync if q % 2 == 0 else nc.gpsimd
                eng.dma_start(out_d[:, qs], outt[:, qs])

    nc.compile()
    return nc


def _round8_up(v):
    return np.ceil(np.asarray(v) * 8.0) / 8.0


def _prep_inputs(X, codewords, scale, fc_w, fc_b):
    X = np.ascontiguousarray(np.asarray(X, np.float32))
    cw = np.asarray(codewords, np.float64)
    sc = np.asarray(scale, np.float64)

    a_hi = sc.astype(np.float32).astype(np.float16)
    b_hi = (-2.0 * sc * cw).astype(np.float32).astype(np.float16)
    cterm = (sc * cw * cw).astype(np.float32)

    # per-channel softmax-invariant shift; capped so e stays under the
    # fp8e4m3 max (240)
    smin = np.maximum(-sc.max(axis=0), 0.0)           # (D,) min_k |scale|
    t_d = np.minimum(TCAP, _round8_up(30.0 * smin)).astype(np.float32)

    cbias = np.zeros((128, NG), np.float32)
    coefT = np.zeros((128, 128 * NG), np.float16)
    selT = np.zeros((128, 128 * NG), ml_dtypes.float8_e4m3)
    cw_8 = cw.astype(np.float32).astype(ml_dtypes.float8_e4m3)
    for g in range(NG):
        for di in range(4):
            d = 4 * g + di
            m = 128 * g + 32 * di + np.arange(K)
            coefT[d, m] = b_hi[:, d]          # pairs v rows (0..63)
            coefT[64 + d, m] = a_hi[:, d]     # pairs u rows (64..127)
            cbias[32 * di + np.arange(K), g] = cterm[:, d] + t_d[d]
            selT[32 * di + np.arange(K), 128 * g + d] = 1.0
            selT[32 * di + np.arange(K), 128 * g + 64 + d] = cw_8[:, d]

    fcwT = np.ascontiguousarray(
        (np.asarray(fc_w, np.float64).T / K).astype(np.float32))
    nfcb = (-np.asarray(fc_b, np.float64)).astype(np.float32).reshape(D, 1)

    Xf = X.reshape(B, D, N)
    in_maps = []
    for core in range(NCORES):
        b, h = core // 2, core % 2
        in_maps.append({
            "x": np.ascontiguousarray(Xf[b, :, h * NL:(h + 1) * NL]),
            "coefT": coefT,
            "selT": selT,
            "cbias": cbias,
            "fcwT": fcwT,
            "nfcb": nfcb,
        })
    return in_maps


_NC = None


def _get_nc():
    global _NC
    if _NC is None:
        _NC = _build_nc()
    return _NC


def run_sharded(X, codewords, scale, fc_w, fc_b, **spmd_kwargs):
    """Build+run; returns (full_output, BassKernelResults)."""
    nc = _get_nc()
    in_maps = _prep_inputs(X, codewords, scale, fc_w, fc_b)
    res = run_bass_kernel_spmd(nc, in_maps, core_ids=list(range(NCORES)),
                               **spmd_kwargs)
    Y = np.empty((B, D, N), np.float32)
    for core in range(NCORES):
        b, h = core // 2, core % 2
        Y[b, :, h * NL:(h + 1) * NL] = res.results[core]["out"].astype(np.float32)
    return Y.reshape(B, D, T, H, W), res


def kernel(X, codewords, scale, fc_w, fc_b):
    Y, _ = run_sharded(X, codewords, scale, fc_w, fc_b)
    return Y


# revision 10
# speedup vs baseline: 1.4961x; 1.2146x over previous
"""Trainium2 Bass kernel for nn_Encoding3D (vq_codebook).

Math per voxel feature x = X[b,d,n] (N = T*H*W):
    logit_k = scale[k,d]*(x - cw[k,d])^2 = a*u + b*v + c   (u=x^2, v=x,
              a=s, b=-2sc, c=sc^2)
    e_k = exp(logit_k + t_d)   (t_d = per-channel softmax-invariant shift)
    E[b,n,d] = x - (sum_k e_k*cw[k,d]) / (sum_k e_k)
    E_glob[b,d] = (1/K) sum_n E;  gamma = sigmoid(E_glob @ fc_w.T + fc_b)
    out = relu(E) * (1 + gamma)          [1+gamma > 0]

Sharding: 8 cores = (b in 0..3) x (N-half in 0..1); the only cross-core
reduction is sum_n E (64 floats) -> AllReduce over core pairs.

Per-core pipeline (4096 voxels, 4 chunks of 1024, 16 channel-groups of 4):
  DVE: basis bt[128, CH] f16 = [v(0:64); u(64:128)] (x is DMA'd into both
       partition halves of xt2 so no partition-shifted engine writes)
  PE:  logits[(4d,k)=128, 512]x2 = coefT_g.T @ bt  (contract all 128 rows;
       coefT has b at row d, a at row 64+d for that column's channel)
  ACT: e = Exp(logits + cbias_g) -> fp8e4m3, written into the paired
       [g|g+1] layout for DoubleRow
  PE:  sums[128, 512]x2 += selT_pair.T @ e  (fp8 DoubleRow: 2 groups per
       matmul at 0.5 cyc/row; s0_d rows 0..63, s1_d rows 64..127)
  DVE: E = x - s1*recip(s0)  (f16, accum egp) ; tail: AllReduce(sum_n E,
       256B) -> gamma -> out = relu(E)*(1+gamma) -> f16 DMA out
"""

import numpy as np
import ml_dtypes

import concourse.bacc as bacc
import concourse.bass as bass
import concourse.mybir as mybir
import concourse.tile as tile
from concourse.bass_utils import run_bass_kernel_spmd

B, D, K = 4, 64, 32
T, H, W = 8, 32, 32
N = T * H * W            # 8192
NCORES = 8
NL = N // 2              # 4096 voxels per core
CH = 1024                # chunk (free-dim) size
NCH = NL // CH           # 4 chunks
NG = D // 4              # 16 groups of 4 channels
f32 = mybir.dt.float32
f16 = mybir.dt.float16
f8 = mybir.dt.float8e4

AF = mybir.ActivationFunctionType
ALU = mybir.AluOpType
DR = mybir.MatmulPerfMode.DoubleRow

TCAP = 5.0               # keeps e = exp(logit+t) <= e^5.x < 240 (fp8 max)


def _build_nc(use_collective=True):
    nc = bacc.Bacc("TRN2", target_bir_lowering=False, debug=False,
                   num_devices=NCORES if use_collective else 1)

    x_d = nc.dram_tensor("x", [D, NL], f32, kind="ExternalInput")
    coefT_d = nc.dram_tensor("coefT", [128, 128 * NG], f16, kind="ExternalInput")
    selT_d = nc.dram_tensor("selT", [128, 128 * NG], f8, kind="ExternalInput")
    cbias_d = nc.dram_tensor("cbias", [128, NG], f32, kind="ExternalInput")
    fcwT_d = nc.dram_tensor("fcwT", [D, D], f32, kind="ExternalInput")
    nfcb_d = nc.dram_tensor("nfcb", [D, 1], f32, kind="ExternalInput")
    out_d = nc.dram_tensor("out", [D, NL], f16, kind="ExternalOutput")

    with tile.TileContext(nc) as tc:
        with (
            tc.tile_pool(name="const", bufs=1) as cpool,
            tc.tile_pool(name="basis", bufs=2) as bpool,
            tc.tile_pool(name="ework", bufs=2) as epool,
            tc.tile_pool(name="fin", bufs=2) as finpool,
            tc.tile_pool(name="persist", bufs=1) as ppool,
            tc.tile_pool(name="psumL", bufs=3, space=bass.MemorySpace.PSUM) as psL,
            tc.tile_pool(name="psumS", bufs=1, space=bass.MemorySpace.PSUM) as psS,
            tc.tile_pool(name="dram", bufs=1, space="DRAM") as dpool,
        ):
            coefT = cpool.tile([128, 128 * NG], f16, tag="coefT")
            selT = cpool.tile([128, 128 * NG], f8, tag="selT")
            cbias = cpool.tile([128, NG], f32, tag="cbias")
            fcwT = cpool.tile([D, D], f32, tag="fcwT")
            nfcb = cpool.tile([D, 1], f32, tag="nfcb")
            wrm = cpool.tile([128, 512], f16, tag="wrm")
            xt2 = ppool.tile([128, NL], f32, tag="xt2")

            # all DMAs on the sync queue so gpsimd (collective) and scalar
            # (exp) never pay queue-drain costs; x chunk 0 first, consts
            # next, remaining x chunks after.
            nc.sync.dma_start(xt2[0:D, 0:CH], x_d[:, 0:CH])
            nc.sync.dma_start(xt2[D:128, 0:CH], x_d[:, 0:CH])
            nc.sync.dma_start(cbias[:], cbias_d[:])
            nc.sync.dma_start(coefT[:], coefT_d[:])
            nc.sync.dma_start(selT[:], selT_d[:])
            for cc_ in range(1, NCH):
                sl = slice(cc_ * CH, (cc_ + 1) * CH)
                nc.sync.dma_start(xt2[0:D, sl], x_d[:, sl])
                nc.sync.dma_start(xt2[D:128, sl], x_d[:, sl])
            nc.sync.dma_start(fcwT[:], fcwT_d[:])
            nc.sync.dma_start(nfcb[:], nfcb_d[:])

            Et = ppool.tile([D, NL], f16, tag="Et")
            egp = ppool.tile([D, NCH], f32, tag="egp")
            cc_in1 = dpool.tile([D, 1], f32, tag="cc_in1")
            cc_out1 = dpool.tile([D, 1], f32, tag="cc_out1")
            cc_in2 = dpool.tile([D, 1], f32, tag="cc_in2")
            cc_out2 = dpool.tile([D, 1], f32, tag="cc_out2")
            Sf1 = ppool.tile([D, 1], f32, tag="Sf1")

            # PE warm-up: dummy matmuls so the clock leaves the idle p-state
            # before the real pipeline starts.
            nc.vector.memset(wrm[:], 0.5)
            warm = psL.tile([128, CH], f32, tag="logits", name="warm")
            for _ in range(6):
                nc.tensor.matmul(warm[:, 0:512], wrm[:, 0:128], wrm[:],
                                 start=True, stop=True)

            def basis_prep(c):
                c0 = c * CH
                bt = bpool.tile([128, CH], f16, tag="bt")
                nc.vector.tensor_copy(bt[0:D, :], xt2[0:D, c0:c0 + CH])
                nc.vector.tensor_tensor(bt[D:128, :], xt2[D:128, c0:c0 + CH],
                                        xt2[D:128, c0:c0 + CH], ALU.mult)
                return bt

            def mm1(c, g):
                logits = psL.tile([128, CH], f32, tag="logits")
                for h in range(2):
                    nc.tensor.matmul(
                        logits[:, 512 * h:512 * (h + 1)],
                        coefT[:, 128 * g:128 * (g + 1)],
                        basis[c][:, 512 * h:512 * (h + 1)],
                        start=True, stop=True)
                return logits

            def finals(c):
                sums = sums_t.pop(c)
                c0 = c * CH
                r = finpool.tile([D, CH], f32, tag="recip")
                nc.vector.reciprocal_approx_fast(r[:], sums[0:D, :])
                corr = finpool.tile([D, CH], f32, tag="corr")
                nc.vector.tensor_tensor(corr[:], sums[D:128, :], r[:], ALU.mult)
                nc.vector.scalar_tensor_tensor(
                    Et[:, c0:c0 + CH], corr[:], -1.0, xt2[0:D, c0:c0 + CH],
                    ALU.mult, ALU.add,
                    accum_out=egp[:, c:c + 1])

            basis = {0: basis_prep(0)}
            sums_t = {}
            units = [(c, g) for c in range(NCH) for g in range(NG)]
            logits_t = {units[0]: mm1(*units[0]), units[1]: mm1(*units[1])}
            et_t = {}
            for i, (c, g) in enumerate(units):
                if g == 0:
                    sums_t[c] = psS.tile([128, CH], f32, tag="sums",
                                         name=f"sums{c}")
                if g == 8 and c + 1 < NCH:
                    basis[c + 1] = basis_prep(c + 1)
                if g % 2 == 0:
                    # paired e layout [128, (j, n)]: j = group within pair;
                    # exp writes are contiguous, the DoubleRow rhs strides
                    et_t[c] = epool.tile([128, 2, CH], f8, tag="et",
                                         name=f"et{c}_{g}")
                et3 = et_t[c]
                nc.scalar.activation(et3[:, g % 2, :],
                                     logits_t.pop((c, g))[:], AF.Exp,
                                     bias=cbias[:, g:g + 1], scale=1.0)
                if i + 2 < len(units):
                    logits_t[units[i + 2]] = mm1(*units[i + 2])
                if g % 2 == 1:
                    p = g // 2
                    selv = selT[:, 256 * p:256 * (p + 1)].rearrange(
                        "p (two m) -> p two m", two=2)
                    for h in range(2):
                        nc.tensor.matmul(
                            sums_t[c][:, 512 * h:512 * (h + 1)],
                            selv,
                            et3[:, :, 512 * h:512 * (h + 1)],
                            start=(g == 1), stop=(g == NG - 1),
                            perf_mode=DR,
                            skip_group_check=True)
                if g == NG - 1:
                    finals(c)
                    if c == NCH - 2 and use_collective:
                        # partial AllReduce over chunks 0..2, hidden under
                        # the last chunk's compute
                        S12 = ppool.tile([D, 1], f32, tag="S12")
                        nc.vector.tensor_reduce(S12[:], egp[:, 0:NCH - 1],
                                                mybir.AxisListType.X, ALU.add)
                        nc.sync.dma_start(cc_in1[:], S12[:])
                        nc.gpsimd.collective_compute(
                            "AllReduce", ALU.add,
                            replica_groups=[[0, 1], [2, 3], [4, 5], [6, 7]],
                            ins=[cc_in1.opt()], outs=[cc_out1.opt()])
                        nc.sync.dma_start(Sf1[:], cc_out1[:])

            # ---- tail: gamma (last chunk's 64-float AllReduce only) ----
            Sf = ppool.tile([D, 1], f32, tag="Sf")
            if use_collective:
                nc.sync.dma_start(cc_in2[:], egp[:, NCH - 1:NCH])
                nc.gpsimd.collective_compute(
                    "AllReduce", ALU.add,
                    replica_groups=[[0, 1], [2, 3], [4, 5], [6, 7]],
                    ins=[cc_in2.opt()], outs=[cc_out2.opt()])
                Sf2 = ppool.tile([D, 1], f32, tag="Sf2")
                nc.sync.dma_start(Sf2[:], cc_out2[:])
                nc.vector.tensor_tensor(Sf[:], Sf1[:], Sf2[:], ALU.add)
            else:
                S = ppool.tile([D, 1], f32, tag="S")
                nc.vector.tensor_reduce(S[:], egp[:, :],
                                        mybir.AxisListType.X, ALU.add)
                nc.sync.dma_start(cc_in2[:], S[:])
                nc.sync.dma_start(Sf[:], cc_in2[:])

            gz = psS.tile([D, 1], f32, tag="sums", name="gz")
            nc.tensor.matmul(gz[:], fcwT[:], Sf[:], start=True, stop=True)
            ue = ppool.tile([D, 1], f32, tag="ue")
            # ue = exp(-z - fcb); gamma = 1/(1+ue)
            nc.scalar.activation(ue[:], gz[:], AF.Exp, bias=nfcb[:, 0:1],
                                 scale=-1.0)
            w1 = ppool.tile([D, 1], f32, tag="w1")
            nc.vector.tensor_scalar_add(w1[:], ue[:], 1.0)
            sg = ppool.tile([D, 1], f32, tag="sg")
            nc.vector.reciprocal(sg[:], w1[:])
            g1 = ppool.tile([D, 1], f32, tag="g1")
            nc.vector.tensor_scalar_add(g1[:], sg[:], 1.0)

            # out = relu(E)*(1+gamma) on DVE per quarter, one output DMA
            # per quarter so store overlaps compute
            outt = ppool.tile([D, NL], f16, tag="outt")
            for q in range(4):
                qs = slice(q * NL // 4, (q + 1) * NL // 4)
                nc.vector.tensor_scalar(outt[:, qs], Et[:, qs],
                                        g1[:, 0:1], 0.0,
                                        ALU.mult, ALU.max)
                nc.sync.dma_start(out_d[:, qs], outt[:, qs])

    nc.compile()
    return nc


def _round8_up(v):
    return np.ceil(np.asarray(v) * 8.0) / 8.0


def _prep_inputs(X, codewords, scale, fc_w, fc_b):
    X = np.ascontiguousarray(np.asarray(X, np.float32))
    cw = np.asarray(codewords, np.float64)
    sc = np.asarray(scale, np.float64)

    a_hi = sc.astype(np.float32).astype(np.float16)
    b_hi = (-2.0 * sc * cw).astype(np.float32).astype(np.float16)
    cterm = (sc * cw * cw).astype(np.float32)

    # per-channel softmax-invariant shift; capped so e stays under the
    # fp8e4m3 max (240)
    smin = np.maximum(-sc.max(axis=0), 0.0)           # (D,) min_k |scale|
    t_d = np.minimum(TCAP, _round8_up(30.0 * smin)).astype(np.float32)

    cbias = np.zeros((128, NG), np.float32)
    coefT = np.zeros((128, 128 * NG), np.float16)
    selT = np.zeros((128, 128 * NG), ml_dtypes.float8_e4m3)
    cw_8 = cw.astype(np.float32).astype(ml_dtypes.float8_e4m3)
    for g in range(NG):
        for di in range(4):
            d = 4 * g + di
            m = 128 * g + 32 * di + np.arange(K)
            coefT[d, m] = b_hi[:, d]          # pairs v rows (0..63)
            coefT[64 + d, m] = a_hi[:, d]     # pairs u rows (64..127)
            cbias[32 * di + np.arange(K), g] = cterm[:, d] + t_d[d]
            selT[32 * di + np.arange(K), 128 * g + d] = 1.0
            selT[32 * di + np.arange(K), 128 * g + 64 + d] = cw_8[:, d]

    fcwT = np.ascontiguousarray(
        (np.asarray(fc_w, np.float64).T / K).astype(np.float32))
    nfcb = (-np.asarray(fc_b, np.float64)).astype(np.float32).reshape(D, 1)

    Xf = X.reshape(B, D, N)
    in_maps = []
    for core in range(NCORES):
        b, h = core // 2, core % 2
        in_maps.append({
            "x": np.ascontiguousarray(Xf[b, :, h * NL:(h + 1) * NL]),
            "coefT": coefT,
            "selT": selT,
            "cbias": cbias,
            "fcwT": fcwT,
            "nfcb": nfcb,
        })
    return in_maps


_NC = None


def _get_nc():
    global _NC
    if _NC is None:
        _NC = _build_nc()
    return _NC


def run_sharded(X, codewords, scale, fc_w, fc_b, **spmd_kwargs):
    """Build+run; returns (full_output, BassKernelResults)."""
    nc = _get_nc()
    in_maps = _prep_inputs(X, codewords, scale, fc_w, fc_b)
    res = run_bass_kernel_spmd(nc, in_maps, core_ids=list(range(NCORES)),
                               **spmd_kwargs)
    Y = np.empty((B, D, N), np.float32)
    for core in range(NCORES):
        b, h = core // 2, core % 2
        Y[b, :, h * NL:(h + 1) * NL] = res.results[core]["out"].astype(np.float32)
    return Y.reshape(B, D, T, H, W), res


def kernel(X, codewords, scale, fc_w, fc_b):
    Y, _ = run_sharded(X, codewords, scale, fc_w, fc_b)
    return Y
